# revision 14
# baseline (speedup 1.0000x reference)
"""NT-Xent (contrastive) loss kernel for Trainium2, 8 NeuronCores.

Data-parallel sharding: core c owns rows [c*1024, (c+1)*1024) of
z = concat(z_i, z_j) (shape [8192, 128]). Every core receives the full z
(the "all-gather" is free on host), normalizes it, computes its row-block
of the similarity matrix sim = (zn @ zn.T) / TEMP via bf16 matmuls, and
reduces each row with a fused exp+accumulate on the scalar engine:

    S_r      = sum_j exp(sim[r, j])
    lse_r    = ln(S_r - exp(sim[r, r]))          (mask the diagonal)
    pos_r    = sim[r, (r + 4096) % 8192]         (row-dot with partner block)
    out[r]   = lse_r - pos_r

Host sums the 8 per-core [128, 8] outputs and divides by 2N.

All scalar-engine functions used are Ln/Exp (one ACT table set): row
norms are computed as rsqrt(x) = exp(-0.5 * ln(x)).
"""

import sys

import numpy as np

if "/opt/trn_rl_repo" not in sys.path:
    sys.path.insert(0, "/opt/trn_rl_repo")

TWO_N = 8192
DIM = 128
N_CORES = 8
RPC = TWO_N // N_CORES  # rows per core = 1024
TEMP = 0.5
N_MTILES = RPC // 128  # 8 M-tiles of 128 rows per core
NCHUNK = 2048  # similarity columns per PSUM round (4 banks)
N_NCHUNKS = TWO_N // NCHUNK  # 4


def _build():
    """Build the SPMD Bass program (same NEFF on all 8 cores; per-core data
    differs via z_blk / z_par inputs)."""
    from contextlib import ExitStack

    import concourse.bass as bass
    import concourse.tile as tile
    from concourse import bacc, masks, mybir

    f32 = mybir.dt.float32
    bf16 = mybir.dt.bfloat16
    AF = mybir.ActivationFunctionType

    nc = bacc.Bacc("TRN2", target_bir_lowering=False, debug=False)
    z_all = nc.dram_tensor("z_all", [TWO_N, DIM], f32, kind="ExternalInput").ap()
    z_blk = nc.dram_tensor("z_blk", [RPC, DIM], f32, kind="ExternalInput").ap()
    z_par = nc.dram_tensor("z_par", [RPC, DIM], f32, kind="ExternalInput").ap()
    out_loss = nc.dram_tensor(
        "row_loss", [128, N_MTILES], f32, kind="ExternalOutput"
    ).ap()

    with tile.TileContext(nc) as tc, ExitStack() as ctx:
        const_pool = ctx.enter_context(tc.tile_pool(name="const", bufs=1))
        ld_pool = ctx.enter_context(tc.tile_pool(name="ld", bufs=4))
        stat_pool = ctx.enter_context(tc.tile_pool(name="stat", bufs=3))
        sq_pool = ctx.enter_context(tc.tile_pool(name="sq", bufs=2))
        rows_pool = ctx.enter_context(tc.tile_pool(name="rows", bufs=1))
        tpose_pool = ctx.enter_context(tc.tile_pool(name="tpose", bufs=1))
        psum_pool = ctx.enter_context(tc.tile_pool(name="psum", bufs=2, space="PSUM"))
        expo_pool = ctx.enter_context(tc.tile_pool(name="expo", bufs=2))

        identity = const_pool.tile([128, 128], bf16, tag="ident")
        masks.make_identity(nc, identity[:])

        # Persistent SBUF tensors.
        znb_all = rows_pool.tile([128, TWO_N], bf16, tag="znb_all")
        znb_blk = rows_pool.tile([128, RPC], bf16, tag="znb_blk")
        znb_par = rows_pool.tile([128, RPC], bf16, tag="znb_par")
        znbT_all = tpose_pool.tile([128, TWO_N], bf16, tag="znbT_all")
        znbT_blk = tpose_pool.tile([128, RPC], bf16, tag="znbT_blk")
        d_vec = tpose_pool.tile([128, N_MTILES], f32, tag="d_vec")
        pos_vec = tpose_pool.tile([128, N_MTILES], f32, tag="pos_vec")
        s_parts = tpose_pool.tile([128, N_MTILES * N_NCHUNKS], f32, tag="s_parts")

        def norm_group(z_src, dst, col0, act_square=False):
            """Normalize one packed 1024-row group: rows a*128+p of z_src
            land at partition p, free cols col0 + a*128 + [0,128). Writes
            sqrt(1/(TEMP*||z||^2))-scaled bf16 rows into dst.

            act_square=True computes the row sum-of-squares on the scalar
            engine (idle during startup) instead of the vector engine."""
            zt = ld_pool.tile([128, 1024], f32, tag="ld")
            nc.sync.dma_start(
                zt[:].rearrange("p (a f) -> p a f", f=128),
                z_src.rearrange("(a p) f -> p a f", p=128),
            )
            ssq = stat_pool.tile([128, 8], f32, tag="ssq")
            if act_square:
                sqs = sq_pool.tile([128, 1024], bf16, tag="sq")
                for a in range(8):
                    nc.scalar.activation(
                        sqs[:, a * 128 : (a + 1) * 128],
                        zt[:, a * 128 : (a + 1) * 128],
                        AF.Square,
                        accum_out=ssq[:, a : a + 1],
                    )
            else:
                sqw = sq_pool.tile([128, 1024], bf16, tag="sq")
                nc.vector.tensor_mul(sqw[:], zt[:], zt[:])
                nc.vector.reduce_sum(
                    ssq[:],
                    sqw[:].rearrange("p (a f) -> p a f", f=128),
                    axis=mybir.AxisListType.X,
                )
            # rn = (TEMP * ssq)^-0.5 = exp(-0.5 * ln(TEMP * ssq))
            lnt = stat_pool.tile([128, 8], f32, tag="lnt")
            nc.scalar.activation(lnt[:], ssq[:], AF.Ln, scale=float(TEMP))
            rn = stat_pool.tile([128, 8], f32, tag="rn")
            nc.scalar.activation(rn[:], lnt[:], AF.Exp, scale=-0.5)
            for a in range(8):
                nc.vector.tensor_scalar_mul(
                    dst[:, col0 + a * 128 : col0 + (a + 1) * 128],
                    zt[:, a * 128 : (a + 1) * 128],
                    rn[:, a : a + 1],
                )

        def transpose_chunk(k):
            """PE-transpose 16 normalized row-tiles into feature-major
            znbT_all[:, k*2048 : (k+1)*2048] via a PSUM bounce."""
            tbf = psum_pool.tile([128, NCHUNK], bf16, tag="mm")
            for t in range(16):
                jt = k * 16 + t
                nc.tensor.transpose(
                    tbf[:, t * 128 : (t + 1) * 128],
                    znb_all[:, jt * 128 : (jt + 1) * 128],
                    identity[:],
                )
            nc.vector.tensor_copy(znbT_all[:, k * NCHUNK : (k + 1) * NCHUNK], tbf[:])

        # --- Prologue: own block, first chunk ------------------------
        norm_group(z_blk, znb_blk, 0, act_square=True)
        norm_group(z_all[0:1024, :], znb_all, 0, act_square=True)
        norm_group(z_all[1024:2048, :], znb_all, 1024, act_square=True)

        tb = psum_pool.tile([128, RPC], bf16, tag="mm")
        for t in range(N_MTILES):
            nc.tensor.transpose(
                tb[:, t * 128 : (t + 1) * 128],
                znb_blk[:, t * 128 : (t + 1) * 128],
                identity[:],
            )
        nc.vector.tensor_copy(znbT_blk[:], tb[:])
        transpose_chunk(0)

        # --- Main loop: normalize/transpose of chunk k+1 is emitted
        # early, spread across chunk k's m-loop, so neither the scalar
        # engine nor the PE starves at chunk boundaries. ---------------
        for k in range(N_NCHUNKS):
            if k + 1 < N_NCHUNKS:
                # Next chunk's normalize lands before this chunk's EXPs in
                # the scalar engine's queue, so its tiny Ln/Exp rnorm ops
                # don't trail the big EXP stream.
                g0 = (k + 1) * 2
                norm_group(z_all[g0 * 1024 : (g0 + 1) * 1024, :], znb_all, g0 * 1024)
                norm_group(
                    z_all[(g0 + 1) * 1024 : (g0 + 2) * 1024, :],
                    znb_all,
                    (g0 + 1) * 1024,
                )
            for m in range(N_MTILES):
                pt = psum_pool.tile([128, NCHUNK], f32, tag="mm")
                for q in range(NCHUNK // 512):
                    nc.tensor.matmul(
                        pt[:, q * 512 : (q + 1) * 512],
                        lhsT=znbT_blk[:, m * 128 : (m + 1) * 128],
                        rhs=znbT_all[
                            :, k * NCHUNK + q * 512 : k * NCHUNK + (q + 1) * 512
                        ],
                        start=True,
                        stop=True,
                    )
                es = expo_pool.tile([128, NCHUNK], bf16, tag="es")
                nc.scalar.activation(
                    es[:],
                    pt[:],
                    AF.Exp,
                    accum_out=s_parts[:, m * N_NCHUNKS + k : m * N_NCHUNKS + k + 1],
                )
                if m == 1 and k + 1 < N_NCHUNKS:
                    transpose_chunk(k + 1)
                if k == 0 and m == 5:
                    # Partner block only feeds the epilogue; keep it off
                    # the startup critical path.
                    norm_group(z_par, znb_par, 0)

        # Diagonal and positive-pair row dots (bf16 products, f32 sums —
        # the diagonal matches what the matmul produces there).
        sqd = sq_pool.tile([128, 1024], bf16, tag="sq")
        nc.vector.tensor_mul(sqd[:], znb_blk[:], znb_blk[:])
        nc.vector.reduce_sum(
            d_vec[:],
            sqd[:].rearrange("p (a f) -> p a f", f=128),
            axis=mybir.AxisListType.X,
        )
        sqp = sq_pool.tile([128, 1024], bf16, tag="sq")
        nc.vector.tensor_mul(sqp[:], znb_blk[:], znb_par[:])
        nc.vector.reduce_sum(
            pos_vec[:],
            sqp[:].rearrange("p (a f) -> p a f", f=128),
            axis=mybir.AxisListType.X,
        )

        # --- Epilogue -------------------------------------------------
        s_tot = stat_pool.tile([128, N_MTILES], f32, tag="s_tot")
        nc.vector.reduce_sum(
            s_tot[:],
            s_parts[:].rearrange("p (m k) -> p m k", k=N_NCHUNKS),
            axis=mybir.AxisListType.X,
        )
        exp_d = stat_pool.tile([128, N_MTILES], f32, tag="exp_d")
        nc.scalar.activation(exp_d[:], d_vec[:], AF.Exp)
        s_excl = stat_pool.tile([128, N_MTILES], f32, tag="s_excl")
        nc.vector.tensor_sub(s_excl[:], s_tot[:], exp_d[:])
        lse = stat_pool.tile([128, N_MTILES], f32, tag="lse")
        nc.scalar.activation(lse[:], s_excl[:], AF.Ln)
        rl = stat_pool.tile([128, N_MTILES], f32, tag="rl")
        nc.vector.tensor_sub(rl[:], lse[:], pos_vec[:])
        nc.sync.dma_start(out_loss, rl[:])

    # Force Ln and Exp onto the single shared ACT table set
    # (natural_log_exp_and_others): the table-load placement pass picks the
    # first set containing each function, which would alternate between
    # exp_and_others and natural_log — one ~1.3us table load per switch.
    import concourse.bacc as bacc_mod
    from concourse.hw_specs import get_activation_tables as _real_gat

    def _gat_ln_exp_shared(arch):
        tabs = _real_gat(arch)
        out = {}
        for name, fns in tabs.items():
            if name != "natural_log_exp_and_others":
                fns = fns - {AF.Ln, AF.Exp}
            out[name] = fns
        return out

    bacc_mod.get_activation_tables = _gat_ln_exp_shared
    try:
        # Runs event-semaphore legalization (splits multi-wait
        # instructions), ACT table loads, and extended-inst ISA codegen.
        nc.compile()
    finally:
        bacc_mod.get_activation_tables = _real_gat
    return nc


_NC_CACHE = None


def _get_nc():
    global _NC_CACHE
    if _NC_CACHE is None:
        _NC_CACHE = _build()
    return _NC_CACHE


def make_in_maps(z_i: np.ndarray, z_j: np.ndarray):
    z = np.concatenate([z_i, z_j], axis=0).astype(np.float32)
    in_maps = []
    for c in range(N_CORES):
        blk0 = c * RPC
        par0 = (c * RPC + TWO_N // 2) % TWO_N
        in_maps.append(
            {
                "z_all": z,
                "z_blk": np.ascontiguousarray(z[blk0 : blk0 + RPC]),
                "z_par": np.ascontiguousarray(z[par0 : par0 + RPC]),
            }
        )
    return in_maps


def kernel(z_i: np.ndarray, z_j: np.ndarray) -> np.ndarray:
    from concourse.bass_utils import run_bass_kernel_spmd

    nc = _get_nc()
    in_maps = make_in_maps(np.asarray(z_i), np.asarray(z_j))
    res = run_bass_kernel_spmd(nc, in_maps, core_ids=list(range(N_CORES)))
    total = 0.0
    for r in res.results:
        total += r["row_loss"].astype(np.float64).sum()
    return np.float32(total / TWO_N)
